# revision 16
# baseline (speedup 1.0000x reference)
"""Two-layer GCN + linear head on 8 Trainium2 NeuronCores (Bass/Tile).

Math (per GCN layer, PyG GCNConv with self loops, symmetric norm):
    deg[c]  = 1 + |{e : col_e == c}|          (self loop counted)
    dinv    = 1/sqrt(deg)
    u       = dinv * (x @ W)                  (row-wise pre-scale)
    out[c]  = sum_{e->c} dinv[c] * u[row_e] + dinv[c]^2 * (x @ W)[c] + b
    x1      = relu(out)

Device mapping:
  - Nodes padded to a multiple of 8*128; dst blocks of 128 nodes sharded
    contiguously across 8 cores (49 blocks/core for N=50000).
  - Dense phase sharded: each core computes u for its own blocks in bf16;
    TWO half AllGathers per layer (fired as soon as each half of the dense
    outputs is ready) build the full bf16 table, split in two DRAM halves
    that each fit the int16 dma_gather index range.
  - Scatter phase: edges sorted by (dst block, src half); per 128-edge tile,
    dma_gather (round-robin over 4 SWDGE queues, issued AHEAD chunks early)
    pulls bf16 u rows; a one-hot matrix O[e,d] = dinv[col_e]*(col_local_e==d)
    is built by DVE in layer 1 and cached to DRAM; layer 2 streams the
    cached one-hots back instead of rebuilding them. One bf16 matmul per
    tile accumulates out^T[f,d] in PSUM across the block's tiles.
  - Self loops never touch DRAM: per block one diag(dinv) matmul against
    the locally kept bf16 u block (u already carries one dinv factor).
  - Transposed accumulation [f,d] feeds layer-2 dense and the head directly
    as matmul stationary operands (no transposes anywhere).

Host does only index-side prep (shard/sort/pad edge lists, integer degree
counts) — all float math runs on device.
"""
import os
import sys

sys.path.insert(0, "/opt/trn_rl_repo")

import numpy as np

import ml_dtypes

P = 128
NCORES = 8
CH = 8       # tiles per dma_gather call (8*128 = 1024 idxs, SWDGE limit)
AHEAD = 3    # gather chunks issued ahead of consumption (per stream)
GBUFS = AHEAD + 2
NQ = 4       # SWDGE queues, round-robin over gather calls
OG = 8       # one-hot tiles per cache group


def _ceil_div(a, b):
    return (a + b - 1) // b


def _prep(x, edge_index):
    """Host-side index prep. Returns per-core input dicts + metadata."""
    N, D = x.shape
    assert D == P
    NB = _ceil_div(N, P)
    NB = _ceil_div(NB, NCORES) * NCORES  # blocks multiple of 8
    Npad = NB * P
    NBC = NB // NCORES
    nbmax = 32768 // (NCORES * P)  # table-half int16 row limit, in blocks
    NBH = [min(nbmax, NBC), NBC - min(nbmax, NBC)]
    HSZ = [NBH[0] * P, NBH[1] * P]  # per-core rows in each half table
    assert NCORES * HSZ[0] <= 32768 and NCORES * HSZ[1] <= 32768

    row = np.asarray(edge_index[0], dtype=np.int64)
    col = np.asarray(edge_index[1], dtype=np.int64)

    deg = np.bincount(col, minlength=Npad).astype(np.int64) + 1  # + self loop
    deg[N:] = 1

    blk = (col >> 7).astype(np.int64)
    core = blk // NBC
    loc = blk % NBC
    csrc = row // (NBC * P)
    j = row - csrc * (NBC * P)
    stream = (j >= HSZ[0]).astype(np.int64)
    gidx = np.where(stream == 0, csrc * HSZ[0] + j, csrc * HSZ[1] + (j - HSZ[0]))

    # order edges by (dst core, dst local block, src half, src)
    order = np.lexsort((gidx, stream, loc, core))
    gidx_s, col_s = gidx[order], col[order]
    core_s, loc_s, str_s = core[order], loc[order], stream[order]

    # per (core, local block, stream) counts -> shared slot table
    key = (core_s * NBC + loc_s) * 2 + str_s
    cnt = np.bincount(key, minlength=NCORES * NBC * 2).reshape(NCORES, NBC, 2)
    slots = _ceil_div(cnt, P).max(axis=0)  # [NBC, 2]
    ntiles = [int(slots[:, s].sum()) for s in (0, 1)]
    tstart = np.zeros((NBC + 1, 2), dtype=np.int64)
    tstart[1:] = np.cumsum(slots, axis=0)

    starts = np.zeros(NCORES * NBC * 2 + 1, dtype=np.int64)
    starts[1:] = np.cumsum(cnt.reshape(-1))

    cores = []
    for c in range(NCORES):
        idx = [np.zeros(max(ntiles[s], 1) * P, dtype=np.int64) for s in (0, 1)]
        colv = [np.full(max(ntiles[s], 1) * P, 999, dtype=np.int64) for s in (0, 1)]
        for i in range(NBC):
            for s in (0, 1):
                k = (c * NBC + i) * 2 + s
                lo, hi = starts[k], starts[k + 1]
                o0 = tstart[i, s] * P
                idx[s][o0 : o0 + hi - lo] = gidx_s[lo:hi]
                colv[s][o0 : o0 + hi - lo] = col_s[lo:hi] & 127
        colstream = np.concatenate([colv[0][: ntiles[0] * P], colv[1][: ntiles[1] * P]])
        degstream = np.ones(colstream.shape[0], dtype=np.float32)
        # per-lane dst-block base: block index of each tile, expanded to lanes
        base = np.concatenate(
            [np.repeat(np.repeat(np.arange(NBC), slots[:, s]) * P, P) for s in (0, 1)]
        )
        m = colstream < P
        degstream[m] = deg[(c * NBC) * P + base[m] + colstream[m]]

        def wrap16(v):  # [ntiles*128] -> [16, n/16] replicated to 128 rows
            w = v.astype(np.int16).reshape(-1, 16).T
            return np.tile(w, (8, 1)).copy()

        def lanes(v, dt):  # [ntiles*128] -> [128, ntiles] (lane-major columns)
            return np.ascontiguousarray(v.reshape(-1, P).T.astype(dt))

        own = slice(c * NBC * P, (c + 1) * NBC * P)
        deg_own = deg[own].astype(np.float32).reshape(NBC, P).T  # [128, NBC]

        xpad = np.zeros((NBC * P, P), dtype=np.float32)
        realn = min(max(N - c * NBC * P, 0), NBC * P)
        xpad[:realn] = x[c * NBC * P : c * NBC * P + realn]

        cores.append(
            dict(
                xT_shard=np.ascontiguousarray(xpad.T).astype(ml_dtypes.bfloat16),
                idxA=wrap16(idx[0]),
                idxB=wrap16(idx[1]),
                colstream=lanes(colstream, np.float32),
                degstream=lanes(degstream, np.float32),
                deg_own=np.ascontiguousarray(deg_own),  # [128, NBC]
            )
        )
    meta = dict(
        N=N, Npad=Npad, NB=NB, NBC=NBC, NBH=NBH, HSZ=HSZ,
        ntilesA=ntiles[0], ntilesB=ntiles[1],
        slots=slots, tstart=tstart,
    )
    return cores, meta


def _build_program(meta, with_bias_gcn):
    """Emit the SPMD bass program (identical for all cores)."""
    from concourse import bacc, mybir
    from concourse.tile import TileContext
    from contextlib import ExitStack

    f32 = mybir.dt.float32
    bf16 = mybir.dt.bfloat16
    i16 = mybir.dt.int16
    i32 = mybir.dt.int32
    AF = mybir.ActivationFunctionType
    OP = mybir.AluOpType

    NBC, NBH, HSZ = meta["NBC"], meta["NBH"], meta["HSZ"]
    nA, nB = meta["ntilesA"], meta["ntilesB"]
    ntiles = [nA, nB]
    slots = meta["slots"]
    tstart = meta["tstart"]
    ncols = nA + nB
    nchunks = [_ceil_div(nA, CH), _ceil_div(nB, CH)]

    nc = bacc.Bacc(
        "TRN2",
        target_bir_lowering=False,
        num_devices=NCORES,
        dynamic_dma_scratch_size=131072,
        num_swdge_queues=NQ,
    )

    xT = nc.declare_dram_parameter("xT_shard", [P, NBC * P], bf16, isOutput=False)
    W1d = nc.declare_dram_parameter("W1", [P, P], bf16, isOutput=False)
    W2d = nc.declare_dram_parameter("W2", [P, P], bf16, isOutput=False)
    Wld = nc.declare_dram_parameter("Wl", [2 * P, P], bf16, isOutput=False)
    b1d = nc.declare_dram_parameter("b1", [1, P], bf16, isOutput=False)
    b2d = nc.declare_dram_parameter("b2", [1, P], bf16, isOutput=False)
    bld = nc.declare_dram_parameter("bl", [1, P], bf16, isOutput=False)
    idxAd = nc.declare_dram_parameter("idxA", [P, max(nA, 1) * 8], i16, isOutput=False)
    idxBd = nc.declare_dram_parameter("idxB", [P, max(nB, 1) * 8], i16, isOutput=False)
    cold = nc.declare_dram_parameter("colstream", [P, ncols], f32, isOutput=False)
    degd = nc.declare_dram_parameter("degstream", [P, ncols], f32, isOutput=False)
    degod = nc.declare_dram_parameter("deg_own", [P, NBC], f32, isOutput=False)
    outd = nc.declare_dram_parameter("out_shard", [NBC * P, P], f32, isOutput=True)

    halves = [h for h in (0, 1) if NBH[h] > 0]
    ag_in = [
        [nc.dram_tensor(f"ag{L}_in_h{h}", [HSZ[h], P], bf16) if NBH[h] else None
         for h in (0, 1)]
        for L in (0, 1)
    ]
    ag_out = [
        [
            nc.dram_tensor(
                f"ag{L}_out_h{h}", [NCORES * HSZ[h], P], bf16, addr_space="Shared"
            ) if NBH[h] else None
            for h in (0, 1)
        ]
        for L in (0, 1)
    ]

    def _emit(tc, ctx):
        const = ctx.enter_context(tc.tile_pool(name="const", bufs=1))
        sb = ctx.enter_context(tc.tile_pool(name="sb", bufs=3))
        gbufs = ctx.enter_context(tc.tile_pool(name="gbufs", bufs=GBUFS))
        obuf = ctx.enter_context(tc.tile_pool(name="obuf", bufs=6))
        psum = ctx.enter_context(tc.tile_pool(name="psum", bufs=4, space="PSUM"))
        psd = ctx.enter_context(tc.tile_pool(name="psd", bufs=2, space="PSUM"))

        # --- constants / streams ---
        iota_i = const.tile([P, P], i32)
        nc.gpsimd.iota(iota_i[:], pattern=[[1, P]], base=0, channel_multiplier=0)
        iota_bf = const.tile([P, P], bf16)
        nc.vector.tensor_copy(out=iota_bf[:], in_=iota_i[:])
        lane_i = const.tile([P, 1], i32)
        nc.gpsimd.iota(lane_i[:], pattern=[[1, 1]], base=0, channel_multiplier=1)
        lane_f = const.tile([P, 1], f32)
        nc.vector.tensor_copy(out=lane_f[:], in_=lane_i[:])

        W1 = const.tile([P, P], bf16)
        W2 = const.tile([P, P], bf16)
        Wl = const.tile([P, 2 * P], bf16)
        nc.sync.dma_start(out=W1[:], in_=W1d[:])
        nc.sync.dma_start(out=W2[:], in_=W2d[:])
        nc.sync.dma_start(out=Wl[:, 0:P], in_=Wld[0:P, :])
        nc.sync.dma_start(out=Wl[:, P : 2 * P], in_=Wld[P : 2 * P, :])

        # bias tiles (row 0 = bias vector), ones row tile
        onesrow = const.tile([P, P], bf16)
        nc.vector.memset(onesrow[:], 0.0)
        nc.vector.memset(onesrow[0:1, :], 1.0)
        btile = []
        for bi, bd in enumerate((b1d, b2d, bld)):
            t = const.tile([P, P], bf16, tag=f"bias{bi}", name=f"bias{bi}")
            nc.vector.memset(t[:], 0.0)
            nc.sync.dma_start(out=t[0:1, :], in_=bd[:])
            btile.append(t)

        idxs = [
            const.tile([P, max(n, 1) * 8], i16, tag=f"idx{s}", name=f"idx{s}")
            for s, n in ((0, nA), (1, nB))
        ]
        nc.sync.dma_start(out=idxs[0][:], in_=idxAd[:])
        nc.sync.dma_start(out=idxs[1][:], in_=idxBd[:])

        colst = const.tile([P, ncols], f32)
        nc.sync.dma_start(out=colst[:], in_=cold[:])
        dinvf = const.tile([P, ncols], f32)
        nc.sync.dma_start(out=dinvf[:], in_=degd[:])
        nc.scalar.activation(out=dinvf[:], in_=dinvf[:], func=AF.Sqrt)
        nc.vector.reciprocal(out=dinvf[:], in_=dinvf[:])

        dinvo = const.tile([P, NBC], f32)
        nc.sync.dma_start(out=dinvo[:], in_=degod[:])
        nc.scalar.activation(out=dinvo[:], in_=dinvo[:], func=AF.Sqrt)
        nc.vector.reciprocal(out=dinvo[:], in_=dinvo[:])

        # persistent per-core tiles
        u_bf = const.tile([P, NBC * P], bf16)  # u blocks [node, f]
        x1T = const.tile([P, NBC * P], bf16)   # x1^T blocks [f, node]

        def dense_block(b, src_lhsT, W, layer):
            """u[b] = dinv_own[b] * (x_b @ W) -> u_bf (bf16, [node, f])."""
            ps = psd.tile([P, P], f32, space="PSUM", tag="psd")
            nc.tensor.matmul(ps[:], lhsT=src_lhsT, rhs=W[:], start=True, stop=True)
            nc.scalar.activation(
                out=u_bf[:, b * P : (b + 1) * P], in_=ps[:], func=AF.Copy,
                scale=dinvo[:, b : b + 1],
            )

        def send_half(layer, h):
            """DMA u_bf half -> ag_in, AllGather into the half table."""
            c0 = 0 if h == 0 else NBH[0]
            nb = NBH[h]
            src = u_bf[:, c0 * P : (c0 + nb) * P].rearrange("p (i f) -> p i f", f=P)
            dst = ag_in[layer][h][:].rearrange("(i p) f -> p i f", p=P)
            nc.sync.dma_start(out=dst, in_=src)
            nc.gpsimd.collective_compute(
                "AllGather", mybir.AluOpType.bypass,
                replica_groups=[list(range(NCORES))],
                ins=[ag_in[layer][h][:]], outs=[ag_out[layer][h][:]],
            )

        qctr = [0]

        def scatter_layer(layer, post_fn):
            """Message passing for one layer; post_fn(b, acc) consumes the
            accumulated transposed block. Gathers are issued AHEAD chunks
            early, round-robin over the SWDGE queues. Layer 0 builds one-hots
            on DVE and caches them to DRAM; layer 1 streams them back."""
            issued = [[], []]  # stream -> list of gbuf tiles

            def ensure(s, cid):
                while len(issued[s]) <= min(cid + AHEAD, nchunks[s] - 1):
                    c0 = len(issued[s])
                    ch = min(CH, ntiles[s] - c0 * CH)
                    g = gbufs.tile([P, CH, P], bf16, tag=f"g{s}")
                    nc.gpsimd.dma_gather(
                        out_ap=g[:, 0:ch, :],
                        in_ap=ag_out[layer][s][:],
                        idxs_ap=idxs[s][:, c0 * CH * 8 : (c0 * CH + ch) * 8],
                        num_idxs=ch * P,
                        num_idxs_reg=ch * P,
                        elem_size=P,
                        queue_num=qctr[0] % NQ,
                    )
                    qctr[0] += 1
                    issued[s].append(g)

            def oh_src(gcol):
                o = obuf.tile([P, P], bf16, tag="oh", name="oh")
                nc.vector.tensor_scalar(
                    out=o[:], in0=iota_bf[:],
                    scalar1=colst[:, gcol : gcol + 1],
                    scalar2=dinvf[:, gcol : gcol + 1],
                    op0=OP.is_equal, op1=OP.mult,
                )
                return o[:]

            for b in range(NBC):
                acc = psum.tile([P, P], f32, space="PSUM", tag="acc")
                # self loop: diag(dinv) against local u block
                od = obuf.tile([P, P], bf16, tag="onehot")
                nc.vector.tensor_scalar(
                    out=od[:], in0=iota_bf[:], scalar1=lane_f[:, 0:1],
                    scalar2=dinvo[:, b : b + 1], op0=OP.is_equal, op1=OP.mult,
                )
                nmm = int(slots[b, 0] + slots[b, 1])
                nc.tensor.matmul(
                    acc[:], lhsT=u_bf[:, b * P : (b + 1) * P], rhs=od[:],
                    start=True, stop=(nmm == 0 and not with_bias_gcn),
                )
                k = 0
                for s in (0, 1):
                    for t in range(tstart[b, s], tstart[b + 1, s]):
                        cid = t // CH
                        ensure(s, cid)
                        g = issued[s][cid]
                        o = oh_src(nA * s + t)
                        k += 1
                        last = (k == nmm) and not with_bias_gcn
                        nc.tensor.matmul(
                            acc[:], lhsT=g[:, t % CH, :], rhs=o,
                            start=False, stop=last,
                        )
                if with_bias_gcn:
                    nc.tensor.matmul(
                        acc[:], lhsT=btile[layer][:], rhs=onesrow[:],
                        start=False, stop=True,
                    )
                post_fn(b, acc)

        phase = os.environ.get("KERNEL_PHASE", "full")

        # ---------- layer 1 dense + half AllGathers ----------
        for b in range(NBC):
            lx = sb.tile([P, P], bf16, tag="xT_in")
            nc.sync.dma_start(out=lx[:], in_=xT[:, b * P : (b + 1) * P])
            dense_block(b, lx[:], W1, 0)
            if b == NBH[0] - 1:
                send_half(0, 0)
        if NBH[1]:
            send_half(0, 1)
        if phase == "dense":
            for b in range(NBC):
                z = sb.tile([P, P], f32, tag="out_t")
                nc.vector.tensor_copy(out=z[:], in_=u_bf[:, b * P : (b + 1) * P])
                nc.sync.dma_start(out=outd[b * P : (b + 1) * P, :], in_=z[:])
            return

        # ---------- layer 1 scatter -> x1T (+ layer 2 dense) ----------
        def post1(b, acc):
            nc.scalar.activation(
                out=x1T[:, b * P : (b + 1) * P], in_=acc[:], func=AF.Relu
            )
            dense_block(b, x1T[:, b * P : (b + 1) * P], W2, 1)
            if b == NBH[0] - 1:
                send_half(1, 0)
            elif b == NBC - 1 and NBH[1]:
                send_half(1, 1)

        scatter_layer(0, post1)
        if phase == "l1":
            for b in range(NBC):
                z = sb.tile([P, P], f32, tag="out_t")
                nc.vector.tensor_copy(out=z[:], in_=x1T[:, b * P : (b + 1) * P])
                nc.sync.dma_start(out=outd[b * P : (b + 1) * P, :], in_=z[:])
            return

        # ---------- layer 2 scatter -> head ----------
        def post2(b, acc):
            x2T = sb.tile([P, P], bf16, tag="x2T")
            nc.scalar.activation(out=x2T[:], in_=acc[:], func=AF.Relu)
            ph = psd.tile([P, P], f32, space="PSUM", tag="ph")
            nc.tensor.matmul(
                ph[:], lhsT=x1T[:, b * P : (b + 1) * P], rhs=Wl[:, 0:P],
                start=True, stop=False,
            )
            nc.tensor.matmul(
                ph[:], lhsT=x2T[:], rhs=Wl[:, P : 2 * P], start=False, stop=False
            )
            nc.tensor.matmul(
                ph[:], lhsT=onesrow[:], rhs=btile[2][:], start=False, stop=True
            )
            ot = sb.tile([P, P], f32, tag="out_t")
            nc.scalar.activation(out=ot[:], in_=ph[:], func=AF.Copy)
            nc.sync.dma_start(out=outd[b * P : (b + 1) * P, :], in_=ot[:])

        scatter_layer(1, post2)

    with TileContext(nc) as tc, ExitStack() as ctx:
        _emit(tc, ctx)

    nc.compile()
    return nc


def kernel(x, edge_index, W1, b1, W2, b2, Wl, bl):
    x = np.asarray(x, dtype=np.float32)
    cores, meta = _prep(x, np.asarray(edge_index))
    with_bias_gcn = bool(np.any(b1) or np.any(b2))

    nc = _build_program(meta, with_bias_gcn)

    bf = ml_dtypes.bfloat16
    shared = dict(
        W1=np.asarray(W1, np.float32).astype(bf),
        W2=np.asarray(W2, np.float32).astype(bf),
        Wl=np.asarray(Wl, np.float32).astype(bf),
        b1=np.asarray(b1, np.float32).astype(bf).reshape(1, P),
        b2=np.asarray(b2, np.float32).astype(bf).reshape(1, P),
        bl=np.asarray(bl, np.float32).astype(bf).reshape(1, P),
    )
    in_maps = [{**c, **shared} for c in cores]
    N = meta["N"]

    if os.environ.get("KERNEL_SIM"):
        from concourse.bass_interp import MultiCoreSim

        sim = MultiCoreSim(nc, NCORES)
        for i in range(NCORES):
            for k, v in in_maps[i].items():
                sim.cores[i].tensor(k)[:] = v
        sim.simulate()
        out = np.concatenate(
            [np.asarray(sim.cores[i].tensor("out_shard")) for i in range(NCORES)],
            axis=0,
        )
        return np.ascontiguousarray(out[:N])

    from concourse.bass_utils import run_bass_kernel_spmd

    trace = bool(int(os.environ.get("KERNEL_TRACE", "0")))
    if trace:
        try:
            import ntff_shim  # noqa: F401
        except ImportError:
            pass

    br = run_bass_kernel_spmd(nc, in_maps, list(range(NCORES)), trace=trace)
    kernel.last_result = br

    out = np.concatenate([r["out_shard"] for r in br.results], axis=0)
    return np.ascontiguousarray(out[:N])
